# revision 42
# baseline (speedup 1.0000x reference)
"""Trainium2 Bass kernel for nn_GPKANLayer (GP-KAN layer forward).

Math (reference):
    psi[b,o,i,m] = vk[o,i] * sqrt(l2/(l2+ex)) * exp(-0.5*(x[b,i]-z[o,i,m])^2/(l2+ex))
    em[b,o,i]   = sum_m psi * q_mu
    ev[b,o,i]   = sum_m psi^2 * (q_var + q_mu^2)
    out1[b,o]   = sum_i em
    out2[b,o]   = sum_i max(ev - em^2, EPS_EDGE)

Fast path (structure verified at runtime): z is a UNIFORM grid shared by
all (o,i), and the lengthscale is one constant.  Let D = l^2 + eps_x,
a = 1/(2D), G[b,i,m] = exp(-a (x[b,i]-z_m)^2).  Then:

    out1[b,o] = sum_{i,m} G[b,i,m] * W1[o,i,m]           (dense matmul)

For out2, the clamp is dropped (it binds with error <= ~1e-6 per entry
on a handful of entries; total effect ~1e-5 relative) and em^2 is
expanded over pairs (m,m').  On a uniform grid the pair Gaussian
collapses onto the HALF-GRID:

    G_m * G_m' = exp(-2a (x - zbar)^2) * exp(-a (z_m - z_m')^2 / 2),
    zbar = (z_m + z_m')/2  in  {z_0, z_0 + d/2, z_0 + d, ...}   (2M-1 pts)

so  sum_i (ev - em^2)  =  G2 . W4even  +  G2h . W4odd   where
    G2  = G*G                    (integer grid,   DVE multiply)
    G2h = G_m * G_{m+1} * const  (half grid,      DVE multiply of a
                                  partition-shifted copy made by DMA)

The device work is therefore: 1 Square + 1 Exp on the scalar engine,
3 cheap fp16 DVE multiplies, 1 small SBUF->SBUF shift DMA, and 48 fp16
matmuls with weights stationary (PSUM-accumulated).  All post-processing
(per-i em/ev, clamp, reductions) is folded into host-precomputed weights.

Sharding: batch dim across 8 cores, params replicated (folded on host).
Outputs come back transposed [O, BLOC]; host reassembles.
"""

import numpy as np

B, O, I, M = 2048, 64, 64, 32
NCORES = 8
BLOC = B // NCORES          # 256 batch rows per core
IB = 4                      # i-values packed per k-chunk (K = IB*M = 128)
NT = I // IB                # 16 k-chunks
GT = 4                      # k-chunks per pipeline group
NG = NT // GT
EPS_XVAR = 1e-06
EPS_QVAR = 1e-05
EPS_VAR = 1e-05
MIN_SCALE = 0.1
EPS_EDGE = 1e-06

# "dve": u/s computed on DVE (frees scalar engine for Exp only)
# "act": s computed on scalar engine via Square with per-partition bias
# "split": alternate whole groups between the two
U_SQUARE_ON = "act"
CSPL = 256                  # columns of each group's square on Act (rest on DVE)
GPS_BUFS = 2                # PSUM bufs for the PE-shift output pool
ACT_GROUPS = 2              # DVE/PE groups covered by one Act slice
PSPL = 256                  # g2i columns on DVE (rest on gpsimd/Pool)

_NC_CACHE = {}


BENCH_UNROLL = 16           # bodies per For_i iteration (amortizes the
                            # all-engine barrier in the loop's reset block)
STAGGERED = False           # staggered semaphore reset in the For_i loop


def _build_nc(repeat=1, unroll=False, variant="full"):
    """Build + compile the per-core Bass program (SPMD, identical on all cores).

    variant: "full" | "nope" (no matmuls/copies) | "nogh" (no shift/gh,
    o1+o2even only) | "peonly" (matmuls+copies only) | "pe16" (o1 chain only)
    """
    import concourse.bass as bass
    import concourse.tile as tile
    from concourse import bacc, mybir

    f32 = mybir.dt.float32
    f16 = mybir.dt.float16
    Exp = mybir.ActivationFunctionType.Exp
    Square = mybir.ActivationFunctionType.Square

    nc = bacc.Bacc("TRN2", target_bir_lowering=False, debug=False)

    xs_d = nc.dram_tensor("xs", [128, NT, BLOC], f16, kind="ExternalInput")
    zn_d = nc.dram_tensor("zn", [128, 1], f32, kind="ExternalInput")
    wg_d = nc.dram_tensor("wg", [128, NT, O], f16, kind="ExternalInput")
    we_d = nc.dram_tensor("we", [128, NT, O], f16, kind="ExternalInput")
    wo_d = nc.dram_tensor("wo", [128, NT, O], f16, kind="ExternalInput")
    ps_d = nc.dram_tensor("ps", [128, 128], f16, kind="ExternalInput")
    out1_d = nc.dram_tensor("out1", [O, BLOC], f32, kind="ExternalOutput")
    out2_d = nc.dram_tensor("out2", [O, BLOC], f32, kind="ExternalOutput")

    with tile.TileContext(nc) as tc:
        with (
            tc.tile_pool(name="const", bufs=1) as cpool,
            tc.tile_pool(name="psum", bufs=2, space="PSUM") as psum,
            tc.tile_pool(name="gpsum", bufs=GPS_BUFS, space="PSUM") as gpsum,
        ):
            frontend = variant in ("full", "nope", "nogh")
            backend = variant != "nope"
            use_gh = variant in ("full", "peonly")
            we_t = wo_t = u_t = s_t = psh_t = g2_t = gh_t = o1s = o2s = None
            xs_t = cpool.tile([128, NT, BLOC], f16, tag="xs")
            zn_t = cpool.tile([128, 1], f32, tag="zn")
            wg_t = cpool.tile([128, NT, O], f16, tag="wg")
            if variant != "pe16":
                we_t = cpool.tile([128, NT, O], f16, tag="we")
            if use_gh:
                wo_t = cpool.tile([128, NT, O], f16, tag="wo")
            if frontend and (U_SQUARE_ON != "act" or CSPL < BLOC):
                u_t = cpool.tile([128, NT, BLOC], f16, tag="u")
            if frontend and U_SQUARE_ON == "act":
                s_t = cpool.tile([128, NT, BLOC], f16, tag="s")
            g_t = cpool.tile([128, NT, BLOC], f16, tag="g")
            if variant == "full":
                psh_t = cpool.tile([128, 128], f16, tag="psh")
            if variant != "pe16":
                g2_t = cpool.tile([128, NT, BLOC], f16, tag="g2")
            if use_gh:
                gh_t = cpool.tile([128, NT, BLOC], f16, tag="gh")
            if backend:
                o1s = cpool.tile([O, BLOC], f32, tag="o1s")
            if backend and variant != "pe16":
                o2s = cpool.tile([O, BLOC], f32, tag="o2s")

            loads = [(xs_d, xs_t), (zn_d, zn_t), (wg_d, wg_t)]
            if we_t is not None:
                loads.append((we_d, we_t))
            if wo_t is not None:
                loads.append((wo_d, wo_t))
            if psh_t is not None:
                loads.append((ps_d, psh_t))
            for d, t in loads:
                nc.sync.dma_start(t[:], d.ap()[:])
            if not frontend:
                # matmul inputs never computed in these variants; keep finite
                nc.vector.memset(g_t[:], 0.25)
                if g2_t is not None:
                    nc.vector.memset(g2_t[:], 0.25)
                if gh_t is not None:
                    nc.vector.memset(gh_t[:], 0.25)

            def emit_body():
                o1p = o2p = None
                if variant != "nope":
                    o1p = psum.tile([O, BLOC], f32, tag="o1p")
                    if variant != "pe16":
                        o2p = psum.tile([O, BLOC], f32, tag="o2p")
                if variant in ("peonly", "pe16"):
                    for tt in range(NT):
                        first = tt == 0
                        last = tt == NT - 1
                        nc.tensor.matmul(o1p[:], wg_t[:, tt], g_t[:, tt],
                                         start=first, stop=last)
                        if variant == "peonly":
                            nc.tensor.matmul(o2p[:], we_t[:, tt], g2_t[:, tt],
                                             start=first, stop=False)
                            nc.tensor.matmul(o2p[:], wo_t[:, tt], gh_t[:, tt],
                                             start=False, stop=last)
                    nc.vector.tensor_scalar_add(o1s[:], o1p[:], 0.0)
                    if variant == "peonly":
                        nc.vector.tensor_scalar_add(o2s[:], o2p[:], 0.0)
                    return
                for g in range(NG):
                    sl = slice(g * GT, (g + 1) * GT)
                    if g % ACT_GROUPS == 0:
                        # Act runs on coarser slices to amortize access latency
                        sla = slice(g * GT, (g + ACT_GROUPS) * GT)
                        if U_SQUARE_ON == "act" and CSPL >= BLOC:
                            # s = (xs + (-z))^2 with per-partition bias
                            nc.scalar.activation(
                                s_t[:, sla], xs_t[:, sla], Square,
                                bias=zn_t[:, :1], scale=1.0,
                            )
                            src = s_t
                        elif U_SQUARE_ON == "act":
                            # column-split: first CSPL cols on Act, rest on DVE
                            nc.scalar.activation(
                                s_t[:, sla, 0:CSPL], xs_t[:, sla, 0:CSPL], Square,
                                bias=zn_t[:, :1], scale=1.0,
                            )
                            nc.vector.tensor_scalar_add(
                                u_t[:, sla, 0:BLOC - CSPL],
                                xs_t[:, sla, CSPL:BLOC], zn_t[:, :1])
                            nc.vector.tensor_mul(
                                s_t[:, sla, CSPL:BLOC],
                                u_t[:, sla, 0:BLOC - CSPL],
                                u_t[:, sla, 0:BLOC - CSPL])
                            src = s_t
                        else:
                            nc.vector.tensor_scalar_add(
                                u_t[:, sla], xs_t[:, sla], zn_t[:, :1])
                            nc.vector.tensor_mul(u_t[:, sla], u_t[:, sla],
                                                 u_t[:, sla])
                            src = u_t
                        nc.scalar.activation(g_t[:, sla], src[:, sla], Exp,
                                             scale=-1.0)
                    if PSPL >= BLOC:
                        nc.vector.tensor_mul(g2_t[:, sl], g_t[:, sl], g_t[:, sl])
                    else:
                        nc.vector.tensor_mul(g2_t[:, sl, 0:PSPL],
                                             g_t[:, sl, 0:PSPL],
                                             g_t[:, sl, 0:PSPL])
                        nc.gpsimd.tensor_mul(g2_t[:, sl, PSPL:BLOC],
                                             g_t[:, sl, PSPL:BLOC],
                                             g_t[:, sl, PSPL:BLOC])
                    if variant == "full":
                        # shifted copy on PE: gsp[k] = g[k+1] (row 127 -> 0)
                        # moving free capped at 512 per matmul (1 PSUM bank)
                        gsp = gpsum.tile([128, GT, BLOC], f32, tag="gsp")
                        hstep = max(1, 512 // BLOC)
                        for h in range(0, GT, hstep):
                            nc.tensor.matmul(
                                gsp[:, h:h + hstep], psh_t[:],
                                g_t[:, g * GT + h:g * GT + h + hstep],
                                start=True, stop=True)
                        nc.vector.tensor_mul(gh_t[:, sl], g_t[:, sl], gsp[:])
                    if variant == "nope":
                        continue
                    for tt in range(g * GT, (g + 1) * GT):
                        first = tt == 0
                        last = tt == NT - 1
                        nc.tensor.matmul(o1p[:], wg_t[:, tt], g_t[:, tt],
                                         start=first, stop=last)
                        nc.tensor.matmul(o2p[:], we_t[:, tt], g2_t[:, tt],
                                         start=first, stop=(last and variant == "nogh"))
                    if variant != "nogh":
                        for tt in range(g * GT, (g + 1) * GT):
                            nc.tensor.matmul(o2p[:], wo_t[:, tt], gh_t[:, tt],
                                             start=False, stop=(tt == NT - 1))
                if variant == "nope":
                    return
                nc.vector.tensor_scalar_add(o1s[:], o1p[:], 0.0)
                nc.vector.tensor_scalar_add(o2s[:], o2p[:], 0.0)

            if repeat == 1:
                emit_body()
            elif unroll:
                for _ in range(repeat):
                    emit_body()
            else:
                u = BENCH_UNROLL if repeat % BENCH_UNROLL == 0 else 1
                with tc.For_i(0, repeat // u, 1, staggered_reset=STAGGERED):
                    for _ in range(u):
                        emit_body()

            if o1s is not None:
                nc.sync.dma_start(out1_d.ap()[:], o1s[:])
            if o2s is not None:
                nc.sync.dma_start(out2_d.ap()[:], o2s[:])

    nc.compile()
    return nc


def _host_weights(zlin, lensq, q_mu, q_log_var, log_variance):
    """Dense fp16 weights in [k=(i4,m), t, o] layout (float64 host math)."""
    f64 = np.float64
    q_mu = np.asarray(q_mu, f64)
    q_var = np.maximum(np.exp(np.asarray(q_log_var, f64)), EPS_QVAR)
    vk = np.maximum(np.exp(np.asarray(log_variance, f64)), EPS_VAR)
    z = np.asarray(zlin, f64)
    D = f64(lensq) + EPS_XVAR
    rho = np.sqrt(f64(lensq) / D)
    c1 = vk * rho                                   # [O, I]
    w1d = c1[:, :, None] * q_mu                     # [O, I, M]
    w2d = (c1 ** 2)[:, :, None] * (q_var + q_mu ** 2)

    # W3[o,i,p], p = m+m' in 0..2M-2: pair expansion of em^2 on half grid
    delta = z[1] - z[0]
    W3 = np.zeros((O, I, 2 * M - 1), f64)
    midx = np.arange(M)
    for m in range(M):
        c = np.exp(-(delta ** 2) * ((m - midx) ** 2) / (4.0 * D))
        W3[:, :, m + midx] += w1d[:, :, m:m + 1] * w1d * c[None, None, :]
    W4even = w2d - W3[:, :, 0::2]                   # [O, I, M]
    W4odd = np.zeros((O, I, M), f64)                # pad m=31 with 0
    W4odd[:, :, :M - 1] = -W3[:, :, 1::2] * np.exp(delta ** 2 / (4.0 * D))

    def blockify(w):                                # [O,I,M] -> [128, NT, O]
        return np.ascontiguousarray(
            w.reshape(O, NT, IB, M).transpose(2, 3, 1, 0).reshape(128, NT, O)
        ).astype(np.float16)

    return blockify(w1d), blockify(W4even), blockify(W4odd)


def _host_prep(x, zlin, lensq, wg, we, wo):
    """Per-core input maps.  x is scaled by s1 on host and cast to fp16."""
    f64 = np.float64
    D = f64(lensq) + EPS_XVAR
    s1 = 1.0 / np.sqrt(2.0 * D)
    z = np.asarray(zlin, f64)
    zn = np.ascontiguousarray(
        np.tile(-s1 * z, IB).reshape(128, 1).astype(np.float32))

    ps = np.eye(128, k=-1, dtype=np.float16)            # out[k] = in[k+1]
    xs = (s1 * np.asarray(x, f64)).astype(np.float16)   # [B, I]
    in_maps = []
    for c in range(NCORES):
        xT = xs[c * BLOC:(c + 1) * BLOC].T              # [I, BLOC]
        arr = xT.reshape(NT, IB, BLOC).transpose(1, 0, 2)   # [i4, t, b]
        x4 = np.ascontiguousarray(
            np.broadcast_to(arr[:, None], (IB, M, NT, BLOC)).reshape(128, NT, BLOC)
        )
        in_maps.append({"xs": x4, "zn": zn, "wg": wg, "we": we, "wo": wo,
                        "ps": ps})
    return in_maps


def _fallback(x, z, q_mu, q_log_var, log_scale, log_variance):
    """Generic numpy implementation (mirrors the reference exactly)."""
    x = np.asarray(x, np.float32)
    q_var = np.maximum(np.exp(np.asarray(q_log_var, np.float32)), EPS_QVAR)
    var_kern = np.maximum(np.exp(np.asarray(log_variance, np.float32)), EPS_VAR)
    lengthscale = np.maximum(np.exp(np.asarray(log_scale, np.float32)), MIN_SCALE)
    ell_sq = lengthscale ** 2
    denom = ell_sq + EPS_XVAR                      # [O, I]
    rho = np.sqrt(ell_sq / denom)
    z = np.asarray(z, np.float32)
    q_mu = np.asarray(q_mu, np.float32)
    w2 = q_var + q_mu ** 2
    nb, no = x.shape[0], z.shape[0]
    o1 = np.empty((nb, no), np.float32)
    o2 = np.empty((nb, no), np.float32)
    for b0 in range(0, nb, 128):
        xs = x[b0:b0 + 128]
        diff = xs[:, None, :, None] - z[None]      # [b, O, I, M]
        psi = (var_kern * rho)[None, :, :, None] * np.exp(
            -0.5 * diff ** 2 / denom[None, :, :, None]
        )
        em = np.einsum("boim,oim->boi", psi, q_mu)
        ev = np.einsum("boim,oim->boi", psi ** 2, w2)
        o1[b0:b0 + 128] = em.sum(2)
        o2[b0:b0 + 128] = np.maximum(ev - em ** 2, EPS_EDGE).sum(2)
    return o1, o2


def _structure(x, z, q_mu, q_log_var, log_scale, log_variance):
    """Return (zlin, lensq) if the fast-path structure holds, else None."""
    if x.shape != (B, I) or z.shape != (O, I, M):
        return None
    z = np.asarray(z)
    if not (z == z[0, 0]).all():
        return None
    zlin = np.asarray(z[0, 0], np.float64)
    d = np.diff(zlin)
    if d[0] <= 0 or not np.allclose(d, d[0], rtol=1e-5, atol=1e-7):
        return None                                 # need a uniform grid
    ls = np.maximum(np.exp(np.asarray(log_scale, np.float32)), np.float32(MIN_SCALE))
    if not (ls == ls.flat[0]).all():
        return None
    return zlin, np.float32(ls.flat[0]) ** 2


def kernel(x, z, q_mu, q_log_var, log_scale, log_variance):
    st = _structure(x, z, q_mu, q_log_var, log_scale, log_variance)
    if st is None:
        return _fallback(x, z, q_mu, q_log_var, log_scale, log_variance)
    zlin, lensq = st

    wg, we, wo = _host_weights(zlin, lensq, q_mu, q_log_var, log_variance)
    in_maps = _host_prep(x, zlin, lensq, wg, we, wo)

    from concourse.bass_utils import run_bass_kernel_spmd

    if "nc" not in _NC_CACHE:
        _NC_CACHE["nc"] = _build_nc(repeat=1)
    nc = _NC_CACHE["nc"]
    res = run_bass_kernel_spmd(nc, in_maps, list(range(NCORES)))
    out1 = np.concatenate(
        [res.results[c]["out1"].T for c in range(NCORES)], 0)
    out2 = np.concatenate(
        [res.results[c]["out2"].T for c in range(NCORES)], 0)
    return np.ascontiguousarray(out1), np.ascontiguousarray(out2)


# revision 44
# speedup vs baseline: 1.0635x; 1.0635x over previous
"""Trainium2 Bass kernel for nn_GPKANLayer (GP-KAN layer forward).

Math (reference):
    psi[b,o,i,m] = vk[o,i] * sqrt(l2/(l2+ex)) * exp(-0.5*(x[b,i]-z[o,i,m])^2/(l2+ex))
    em[b,o,i]   = sum_m psi * q_mu
    ev[b,o,i]   = sum_m psi^2 * (q_var + q_mu^2)
    out1[b,o]   = sum_i em
    out2[b,o]   = sum_i max(ev - em^2, EPS_EDGE)

Fast path (structure verified at runtime): z is a UNIFORM grid shared by
all (o,i), and the lengthscale is one constant.  Let D = l^2 + eps_x,
a = 1/(2D), G[b,i,m] = exp(-a (x[b,i]-z_m)^2).  Then:

    out1[b,o] = sum_{i,m} G[b,i,m] * W1[o,i,m]           (dense matmul)

For out2, the clamp is dropped (it binds with error <= ~1e-6 per entry
on a handful of entries; total effect ~1e-5 relative) and em^2 is
expanded over pairs (m,m').  On a uniform grid the pair Gaussian
collapses onto the HALF-GRID:

    G_m * G_m' = exp(-2a (x - zbar)^2) * exp(-a (z_m - z_m')^2 / 2),
    zbar = (z_m + z_m')/2  in  {z_0, z_0 + d/2, z_0 + d, ...}   (2M-1 pts)

so  sum_i (ev - em^2)  =  G2 . W4even  +  G2h . W4odd   where
    G2  = G*G                    (integer grid, DVE multiply)
    G2h = G_m * G_{m+1} * const  (half grid:  PE shift-permutation matmul
                                  makes G_{k+1} in PSUM, DVE multiplies)

The device work per body is therefore: 1 Square + 1 Exp on the scalar
engine (per-partition bias folds the z subtraction into the Square),
2 fp16 DVE multiplies + 2 PSUM copies, 8 shift matmuls, and 48 fp16
matmuls with weights stationary (PSUM-accumulated).  All post-processing
(per-i em/ev, clamp, reductions) is folded into host-precomputed weights.

HW notes (measured on axon trn2): tc.For_i pays a ~33us all-engine
barrier + semaphore reset per iteration, so benchmark builds unroll
BENCH_UNROLL bodies per iteration; partition-shifted SBUF->SBUF DMA
costs ~16us per transfer (hence the PE shift instead); matmuls cost
~75ns each; DVE fp16 SBUF ops run at 2x.

Sharding: batch dim across 8 cores, params replicated (folded on host).
Outputs come back transposed [O, BLOC]; host reassembles.
"""

import numpy as np

B, O, I, M = 2048, 64, 64, 32
NCORES = 8
BLOC = B // NCORES          # 256 batch rows per core
IB = 4                      # i-values packed per k-chunk (K = IB*M = 128)
NT = I // IB                # 16 k-chunks
GT = 4                      # k-chunks per pipeline group
NG = NT // GT
EPS_XVAR = 1e-06
EPS_QVAR = 1e-05
EPS_VAR = 1e-05
MIN_SCALE = 0.1
EPS_EDGE = 1e-06

# "dve": u/s computed on DVE (frees scalar engine for Exp only)
# "act": s computed on scalar engine via Square with per-partition bias
# "split": alternate whole groups between the two
U_SQUARE_ON = "act"
CSPL = 256                  # columns of each group's square on Act (rest on DVE)
GPS_BUFS = 2                # PSUM bufs for the PE-shift output pool
ACT_GROUPS = 1              # DVE/PE groups covered by one Act slice
PSPL = 256                  # g2i columns on DVE (rest on gpsimd/Pool)

_NC_CACHE = {}


BENCH_UNROLL = 16           # bodies per For_i iteration (amortizes the
                            # all-engine barrier in the loop's reset block)
STAGGERED = False           # staggered semaphore reset in the For_i loop


def _build_nc(repeat=1, unroll=False, variant="full"):
    """Build + compile the per-core Bass program (SPMD, identical on all cores).

    variant: "full" | "nope" (no matmuls/copies) | "nogh" (no shift/gh,
    o1+o2even only) | "peonly" (matmuls+copies only) | "pe16" (o1 chain only)
    """
    import concourse.bass as bass
    import concourse.tile as tile
    from concourse import bacc, mybir

    f32 = mybir.dt.float32
    f16 = mybir.dt.float16
    Exp = mybir.ActivationFunctionType.Exp
    Square = mybir.ActivationFunctionType.Square

    nc = bacc.Bacc("TRN2", target_bir_lowering=False, debug=False)

    xs_d = nc.dram_tensor("xs", [128, NT, BLOC], f16, kind="ExternalInput")
    zn_d = nc.dram_tensor("zn", [128, 1], f32, kind="ExternalInput")
    wg_d = nc.dram_tensor("wg", [128, NT, O], f16, kind="ExternalInput")
    we_d = nc.dram_tensor("we", [128, NT, O], f16, kind="ExternalInput")
    wo_d = nc.dram_tensor("wo", [128, NT, O], f16, kind="ExternalInput")
    ps_d = nc.dram_tensor("ps", [128, 128], f16, kind="ExternalInput")
    out1_d = nc.dram_tensor("out1", [O, BLOC], f32, kind="ExternalOutput")
    out2_d = nc.dram_tensor("out2", [O, BLOC], f32, kind="ExternalOutput")

    with tile.TileContext(nc) as tc:
        with (
            tc.tile_pool(name="const", bufs=1) as cpool,
            tc.tile_pool(name="psum", bufs=2, space="PSUM") as psum,
            tc.tile_pool(name="gpsum", bufs=GPS_BUFS, space="PSUM") as gpsum,
        ):
            frontend = variant in ("full", "nope", "nogh")
            backend = variant != "nope"
            use_gh = variant in ("full", "peonly")
            we_t = wo_t = u_t = s_t = psh_t = g2_t = gh_t = o1s = o2s = None
            xs_t = cpool.tile([128, NT, BLOC], f16, tag="xs")
            zn_t = cpool.tile([128, 1], f32, tag="zn")
            wg_t = cpool.tile([128, NT, O], f16, tag="wg")
            if variant != "pe16":
                we_t = cpool.tile([128, NT, O], f16, tag="we")
            if use_gh:
                wo_t = cpool.tile([128, NT, O], f16, tag="wo")
            if frontend and (U_SQUARE_ON != "act" or CSPL < BLOC):
                u_t = cpool.tile([128, NT, BLOC], f16, tag="u")
            if frontend and U_SQUARE_ON == "act":
                s_t = cpool.tile([128, NT, BLOC], f16, tag="s")
            g_t = cpool.tile([128, NT, BLOC], f16, tag="g")
            if variant == "full":
                psh_t = cpool.tile([128, 128], f16, tag="psh")
            if variant != "pe16":
                g2_t = cpool.tile([128, NT, BLOC], f16, tag="g2")
            if use_gh:
                gh_t = cpool.tile([128, NT, BLOC], f16, tag="gh")
            if backend:
                o1s = cpool.tile([O, BLOC], f32, tag="o1s")
            if backend and variant != "pe16":
                o2s = cpool.tile([O, BLOC], f32, tag="o2s")

            loads = [(xs_d, xs_t), (zn_d, zn_t), (wg_d, wg_t)]
            if we_t is not None:
                loads.append((we_d, we_t))
            if wo_t is not None:
                loads.append((wo_d, wo_t))
            if psh_t is not None:
                loads.append((ps_d, psh_t))
            for d, t in loads:
                nc.sync.dma_start(t[:], d.ap()[:])
            if not frontend:
                # matmul inputs never computed in these variants; keep finite
                nc.vector.memset(g_t[:], 0.25)
                if g2_t is not None:
                    nc.vector.memset(g2_t[:], 0.25)
                if gh_t is not None:
                    nc.vector.memset(gh_t[:], 0.25)

            def emit_body():
                o1p = o2p = None
                if variant != "nope":
                    o1p = psum.tile([O, BLOC], f32, tag="o1p")
                    if variant != "pe16":
                        o2p = psum.tile([O, BLOC], f32, tag="o2p")
                if variant in ("peonly", "pe16"):
                    for tt in range(NT):
                        first = tt == 0
                        last = tt == NT - 1
                        nc.tensor.matmul(o1p[:], wg_t[:, tt], g_t[:, tt],
                                         start=first, stop=last)
                        if variant == "peonly":
                            nc.tensor.matmul(o2p[:], we_t[:, tt], g2_t[:, tt],
                                             start=first, stop=False)
                            nc.tensor.matmul(o2p[:], wo_t[:, tt], gh_t[:, tt],
                                             start=False, stop=last)
                    nc.vector.tensor_scalar_add(o1s[:], o1p[:], 0.0)
                    if variant == "peonly":
                        nc.vector.tensor_scalar_add(o2s[:], o2p[:], 0.0)
                    return
                for g in range(NG):
                    sl = slice(g * GT, (g + 1) * GT)
                    if g % ACT_GROUPS == 0:
                        # Act runs on coarser slices to amortize access latency
                        sla = slice(g * GT, (g + ACT_GROUPS) * GT)
                        if U_SQUARE_ON == "act" and CSPL >= BLOC:
                            # s = (xs + (-z))^2 with per-partition bias
                            nc.scalar.activation(
                                s_t[:, sla], xs_t[:, sla], Square,
                                bias=zn_t[:, :1], scale=1.0,
                            )
                            src = s_t
                        elif U_SQUARE_ON == "act":
                            # column-split: first CSPL cols on Act, rest on DVE
                            nc.scalar.activation(
                                s_t[:, sla, 0:CSPL], xs_t[:, sla, 0:CSPL], Square,
                                bias=zn_t[:, :1], scale=1.0,
                            )
                            nc.vector.tensor_scalar_add(
                                u_t[:, sla, 0:BLOC - CSPL],
                                xs_t[:, sla, CSPL:BLOC], zn_t[:, :1])
                            nc.vector.tensor_mul(
                                s_t[:, sla, CSPL:BLOC],
                                u_t[:, sla, 0:BLOC - CSPL],
                                u_t[:, sla, 0:BLOC - CSPL])
                            src = s_t
                        else:
                            nc.vector.tensor_scalar_add(
                                u_t[:, sla], xs_t[:, sla], zn_t[:, :1])
                            nc.vector.tensor_mul(u_t[:, sla], u_t[:, sla],
                                                 u_t[:, sla])
                            src = u_t
                        nc.scalar.activation(g_t[:, sla], src[:, sla], Exp,
                                             scale=-1.0)
                    if PSPL >= BLOC:
                        nc.vector.tensor_mul(g2_t[:, sl], g_t[:, sl], g_t[:, sl])
                    else:
                        nc.vector.tensor_mul(g2_t[:, sl, 0:PSPL],
                                             g_t[:, sl, 0:PSPL],
                                             g_t[:, sl, 0:PSPL])
                        nc.gpsimd.tensor_mul(g2_t[:, sl, PSPL:BLOC],
                                             g_t[:, sl, PSPL:BLOC],
                                             g_t[:, sl, PSPL:BLOC])
                    if variant == "full":
                        # shifted copy on PE: gsp[k] = g[k+1] (row 127 -> 0)
                        # moving free capped at 512 per matmul (1 PSUM bank)
                        gsp = gpsum.tile([128, GT, BLOC], f32, tag="gsp")
                        hstep = max(1, 512 // BLOC)
                        for h in range(0, GT, hstep):
                            nc.tensor.matmul(
                                gsp[:, h:h + hstep], psh_t[:],
                                g_t[:, g * GT + h:g * GT + h + hstep],
                                start=True, stop=True)
                        nc.vector.tensor_mul(gh_t[:, sl], g_t[:, sl], gsp[:])
                    if variant == "nope":
                        continue
                    for tt in range(g * GT, (g + 1) * GT):
                        first = tt == 0
                        last = tt == NT - 1
                        nc.tensor.matmul(o1p[:], wg_t[:, tt], g_t[:, tt],
                                         start=first, stop=last)
                        nc.tensor.matmul(o2p[:], we_t[:, tt], g2_t[:, tt],
                                         start=first, stop=(last and variant == "nogh"))
                    if variant != "nogh":
                        for tt in range(g * GT, (g + 1) * GT):
                            nc.tensor.matmul(o2p[:], wo_t[:, tt], gh_t[:, tt],
                                             start=False, stop=(tt == NT - 1))
                if variant == "nope":
                    return
                nc.vector.tensor_scalar_add(o1s[:], o1p[:], 0.0)
                nc.vector.tensor_scalar_add(o2s[:], o2p[:], 0.0)

            if repeat == 1:
                emit_body()
            elif unroll:
                for _ in range(repeat):
                    emit_body()
            else:
                u = BENCH_UNROLL if repeat % BENCH_UNROLL == 0 else 1
                with tc.For_i(0, repeat // u, 1, staggered_reset=STAGGERED):
                    for _ in range(u):
                        emit_body()

            if o1s is not None:
                nc.sync.dma_start(out1_d.ap()[:], o1s[:])
            if o2s is not None:
                nc.sync.dma_start(out2_d.ap()[:], o2s[:])

    nc.compile()
    return nc


def _host_weights(zlin, lensq, q_mu, q_log_var, log_variance):
    """Dense fp16 weights in [k=(i4,m), t, o] layout (float64 host math)."""
    f64 = np.float64
    q_mu = np.asarray(q_mu, f64)
    q_var = np.maximum(np.exp(np.asarray(q_log_var, f64)), EPS_QVAR)
    vk = np.maximum(np.exp(np.asarray(log_variance, f64)), EPS_VAR)
    z = np.asarray(zlin, f64)
    D = f64(lensq) + EPS_XVAR
    rho = np.sqrt(f64(lensq) / D)
    c1 = vk * rho                                   # [O, I]
    w1d = c1[:, :, None] * q_mu                     # [O, I, M]
    w2d = (c1 ** 2)[:, :, None] * (q_var + q_mu ** 2)

    # W3[o,i,p], p = m+m' in 0..2M-2: pair expansion of em^2 on half grid
    delta = z[1] - z[0]
    W3 = np.zeros((O, I, 2 * M - 1), f64)
    midx = np.arange(M)
    for m in range(M):
        c = np.exp(-(delta ** 2) * ((m - midx) ** 2) / (4.0 * D))
        W3[:, :, m + midx] += w1d[:, :, m:m + 1] * w1d * c[None, None, :]
    W4even = w2d - W3[:, :, 0::2]                   # [O, I, M]
    W4odd = np.zeros((O, I, M), f64)                # pad m=31 with 0
    W4odd[:, :, :M - 1] = -W3[:, :, 1::2] * np.exp(delta ** 2 / (4.0 * D))

    def blockify(w):                                # [O,I,M] -> [128, NT, O]
        return np.ascontiguousarray(
            w.reshape(O, NT, IB, M).transpose(2, 3, 1, 0).reshape(128, NT, O)
        ).astype(np.float16)

    return blockify(w1d), blockify(W4even), blockify(W4odd)


def _host_prep(x, zlin, lensq, wg, we, wo):
    """Per-core input maps.  x is scaled by s1 on host and cast to fp16."""
    f64 = np.float64
    D = f64(lensq) + EPS_XVAR
    s1 = 1.0 / np.sqrt(2.0 * D)
    z = np.asarray(zlin, f64)
    zn = np.ascontiguousarray(
        np.tile(-s1 * z, IB).reshape(128, 1).astype(np.float32))

    ps = np.eye(128, k=-1, dtype=np.float16)            # out[k] = in[k+1]
    xs = (s1 * np.asarray(x, f64)).astype(np.float16)   # [B, I]
    in_maps = []
    for c in range(NCORES):
        xT = xs[c * BLOC:(c + 1) * BLOC].T              # [I, BLOC]
        arr = xT.reshape(NT, IB, BLOC).transpose(1, 0, 2)   # [i4, t, b]
        x4 = np.ascontiguousarray(
            np.broadcast_to(arr[:, None], (IB, M, NT, BLOC)).reshape(128, NT, BLOC)
        )
        in_maps.append({"xs": x4, "zn": zn, "wg": wg, "we": we, "wo": wo,
                        "ps": ps})
    return in_maps


def _fallback(x, z, q_mu, q_log_var, log_scale, log_variance):
    """Generic numpy implementation (mirrors the reference exactly)."""
    x = np.asarray(x, np.float32)
    q_var = np.maximum(np.exp(np.asarray(q_log_var, np.float32)), EPS_QVAR)
    var_kern = np.maximum(np.exp(np.asarray(log_variance, np.float32)), EPS_VAR)
    lengthscale = np.maximum(np.exp(np.asarray(log_scale, np.float32)), MIN_SCALE)
    ell_sq = lengthscale ** 2
    denom = ell_sq + EPS_XVAR                      # [O, I]
    rho = np.sqrt(ell_sq / denom)
    z = np.asarray(z, np.float32)
    q_mu = np.asarray(q_mu, np.float32)
    w2 = q_var + q_mu ** 2
    nb, no = x.shape[0], z.shape[0]
    o1 = np.empty((nb, no), np.float32)
    o2 = np.empty((nb, no), np.float32)
    for b0 in range(0, nb, 128):
        xs = x[b0:b0 + 128]
        diff = xs[:, None, :, None] - z[None]      # [b, O, I, M]
        psi = (var_kern * rho)[None, :, :, None] * np.exp(
            -0.5 * diff ** 2 / denom[None, :, :, None]
        )
        em = np.einsum("boim,oim->boi", psi, q_mu)
        ev = np.einsum("boim,oim->boi", psi ** 2, w2)
        o1[b0:b0 + 128] = em.sum(2)
        o2[b0:b0 + 128] = np.maximum(ev - em ** 2, EPS_EDGE).sum(2)
    return o1, o2


def _structure(x, z, q_mu, q_log_var, log_scale, log_variance):
    """Return (zlin, lensq) if the fast-path structure holds, else None."""
    if x.shape != (B, I) or z.shape != (O, I, M):
        return None
    z = np.asarray(z)
    if not (z == z[0, 0]).all():
        return None
    zlin = np.asarray(z[0, 0], np.float64)
    d = np.diff(zlin)
    if d[0] <= 0 or not np.allclose(d, d[0], rtol=1e-5, atol=1e-7):
        return None                                 # need a uniform grid
    ls = np.maximum(np.exp(np.asarray(log_scale, np.float32)), np.float32(MIN_SCALE))
    if not (ls == ls.flat[0]).all():
        return None
    return zlin, np.float32(ls.flat[0]) ** 2


def kernel(x, z, q_mu, q_log_var, log_scale, log_variance):
    st = _structure(x, z, q_mu, q_log_var, log_scale, log_variance)
    if st is None:
        return _fallback(x, z, q_mu, q_log_var, log_scale, log_variance)
    zlin, lensq = st

    wg, we, wo = _host_weights(zlin, lensq, q_mu, q_log_var, log_variance)
    in_maps = _host_prep(x, zlin, lensq, wg, we, wo)

    from concourse.bass_utils import run_bass_kernel_spmd

    if "nc" not in _NC_CACHE:
        _NC_CACHE["nc"] = _build_nc(repeat=1)
    nc = _NC_CACHE["nc"]
    res = run_bass_kernel_spmd(nc, in_maps, list(range(NCORES)))
    out1 = np.concatenate(
        [res.results[c]["out1"].T for c in range(NCORES)], 0)
    out2 = np.concatenate(
        [res.results[c]["out2"].T for c in range(NCORES)], 0)
    return np.ascontiguousarray(out1), np.ascontiguousarray(out2)


# revision 53
# speedup vs baseline: 1.1065x; 1.0404x over previous
"""Trainium2 Bass kernel for nn_GPKANLayer (GP-KAN layer forward).

Math (reference):
    psi[b,o,i,m] = vk[o,i] * sqrt(l2/(l2+ex)) * exp(-0.5*(x[b,i]-z[o,i,m])^2/(l2+ex))
    em[b,o,i]   = sum_m psi * q_mu
    ev[b,o,i]   = sum_m psi^2 * (q_var + q_mu^2)
    out1[b,o]   = sum_i em
    out2[b,o]   = sum_i max(ev - em^2, EPS_EDGE)

Fast path (structure verified at runtime): z is a UNIFORM grid shared by
all (o,i), and the lengthscale is one constant.  Let D = l^2 + eps_x,
a = 1/(2D), G[b,i,m] = exp(-a (x[b,i]-z_m)^2).  Then:

    out1[b,o] = sum_{i,m} G[b,i,m] * W1[o,i,m]           (dense matmul)

For out2, the clamp is dropped (it binds with error <= ~1e-6 per entry
on a handful of entries; total effect ~1e-5 relative) and em^2 is
expanded over pairs (m,m').  On a uniform grid the pair Gaussian
collapses onto the HALF-GRID:

    G_m * G_m' = exp(-2a (x - zbar)^2) * exp(-a (z_m - z_m')^2 / 2),
    zbar = (z_m + z_m')/2  in  {z_0, z_0 + d/2, z_0 + d, ...}   (2M-1 pts)

so  sum_i (ev - em^2)  =  G2 . W4even  +  G2h . W4odd   where
    G2  = G*G                    (integer grid, DVE multiply)
    G2h = G_m * G_{m+1} * const  (half grid:  PE shift-permutation matmul
                                  makes G_{k+1} in PSUM, DVE multiplies)

The device work per body is therefore: 1 Square + 1 Exp on the scalar
engine (per-partition bias folds the z subtraction into the Square),
2 fp16 DVE multiplies + 2 PSUM copies, 8 shift matmuls, and 48 fp16
matmuls with weights stationary (PSUM-accumulated).  All post-processing
(per-i em/ev, clamp, reductions) is folded into host-precomputed weights.

HW notes (measured on axon trn2): tc.For_i pays a ~33us all-engine
barrier + semaphore reset per iteration, so benchmark builds unroll
BENCH_UNROLL bodies per iteration; partition-shifted SBUF->SBUF DMA
costs ~16us per transfer (hence the PE shift instead); matmuls cost
~75ns each; DVE fp16 SBUF ops run at 2x.

Sharding: batch dim across 8 cores, params replicated (folded on host).
Outputs come back transposed [O, BLOC]; host reassembles.
"""

import numpy as np

B, O, I, M = 2048, 64, 64, 32
NCORES = 8
BLOC = B // NCORES          # 256 batch rows per core
IB = 4                      # i-values packed per k-chunk (K = IB*M = 128)
NT = I // IB                # 16 k-chunks
GT = 4                      # k-chunks per pipeline group
NG = NT // GT
EPS_XVAR = 1e-06
EPS_QVAR = 1e-05
EPS_VAR = 1e-05
MIN_SCALE = 0.1
EPS_EDGE = 1e-06

# "dve": u/s computed on DVE (frees scalar engine for Exp only)
# "act": s computed on scalar engine via Square with per-partition bias
# "split": alternate whole groups between the two
U_SQUARE_ON = "act"
SQ_DVE_LAST = 0             # trailing groups whose square runs on DVE (u+s)
CSPL = 256                  # columns of each group's square on Act (rest on DVE)
GPS_BUFS = 2                # PSUM bufs for the PE-shift output pool
ACT_GROUPS = 1              # DVE/PE groups covered by one Act slice
PSPL = 256                  # g2i columns on DVE (rest on gpsimd/Pool)
WORK_BUFS = 2               # double-buffer work tiles across bodies

_NC_CACHE = {}


BENCH_UNROLL = 16           # bodies per For_i iteration (amortizes the
                            # all-engine barrier in the loop's reset block)
STAGGERED = False           # staggered semaphore reset in the For_i loop


def _build_nc(repeat=1, unroll=False, variant="full"):
    """Build + compile the per-core Bass program (SPMD, identical on all cores).

    variant: "full" | "nope" (no matmuls/copies) | "nogh" (no shift/gh,
    o1+o2even only) | "peonly" (matmuls+copies only) | "pe16" (o1 chain only)
    """
    import concourse.bass as bass
    import concourse.tile as tile
    from concourse import bacc, mybir

    f32 = mybir.dt.float32
    f16 = mybir.dt.float16
    Exp = mybir.ActivationFunctionType.Exp
    Square = mybir.ActivationFunctionType.Square

    nc = bacc.Bacc("TRN2", target_bir_lowering=False, debug=False)

    xs_d = nc.dram_tensor("xs", [128, NT, BLOC], f16, kind="ExternalInput")
    zn_d = nc.dram_tensor("zn", [128, 1], f32, kind="ExternalInput")
    wg_d = nc.dram_tensor("wg", [128, NT, O], f16, kind="ExternalInput")
    we_d = nc.dram_tensor("we", [128, NT, O], f16, kind="ExternalInput")
    wo_d = nc.dram_tensor("wo", [128, NT, O], f16, kind="ExternalInput")
    ps_d = nc.dram_tensor("ps", [128, 128], f16, kind="ExternalInput")
    out1_d = nc.dram_tensor("out1", [O, BLOC], f32, kind="ExternalOutput")
    out2_d = nc.dram_tensor("out2", [O, BLOC], f32, kind="ExternalOutput")

    with tile.TileContext(nc) as tc:
        with (
            tc.tile_pool(name="const", bufs=1) as cpool,
            tc.tile_pool(name="work", bufs=WORK_BUFS) as work,
            tc.tile_pool(name="psum", bufs=2, space="PSUM") as psum,
            tc.tile_pool(name="gpsum", bufs=GPS_BUFS, space="PSUM") as gpsum,
        ):
            frontend = variant in ("full", "nope", "nogh")
            backend = variant != "nope"
            use_gh = variant in ("full", "peonly")
            we_t = wo_t = u_t = s_t = psh_t = g2_t = gh_t = o1s = o2s = None
            xs_t = cpool.tile([128, NT, BLOC], f16, tag="xs")
            zn_t = cpool.tile([128, 1], f32, tag="zn")
            wg_t = cpool.tile([128, NT, O], f16, tag="wg")
            if variant != "pe16":
                we_t = cpool.tile([128, NT, O], f16, tag="we")
            if use_gh:
                wo_t = cpool.tile([128, NT, O], f16, tag="wo")
            need_u = U_SQUARE_ON != "act" or CSPL < BLOC or SQ_DVE_LAST > 0
            if variant != "full":
                if frontend and need_u:
                    u_t = cpool.tile([128, NT, BLOC], f16, tag="u")
                if frontend and U_SQUARE_ON == "act":
                    s_t = cpool.tile([128, NT, BLOC], f16, tag="s")
            if variant == "full":
                g_t = None  # allocated per body from the work pool
            else:
                g_t = cpool.tile([128, NT, BLOC], f16, tag="g")
            if variant == "full":
                psh_t = cpool.tile([128, 128], f16, tag="psh")
            if variant not in ("pe16", "full"):
                g2_t = cpool.tile([128, NT, BLOC], f16, tag="g2")
            if use_gh and variant != "full":
                gh_t = cpool.tile([128, NT, BLOC], f16, tag="gh")
            if backend:
                o1s = cpool.tile([O, BLOC], f32, tag="o1s")
            if backend and variant != "pe16":
                o2s = cpool.tile([O, BLOC], f32, tag="o2s")

            loads = [(xs_d, xs_t), (zn_d, zn_t), (wg_d, wg_t)]
            if we_t is not None:
                loads.append((we_d, we_t))
            if wo_t is not None:
                loads.append((wo_d, wo_t))
            if psh_t is not None:
                loads.append((ps_d, psh_t))
            for d, t in loads:
                nc.sync.dma_start(t[:], d.ap()[:])
            if not frontend:
                # matmul inputs never computed in these variants; keep finite
                nc.vector.memset(g_t[:], 0.25)
                if g2_t is not None:
                    nc.vector.memset(g2_t[:], 0.25)
                if gh_t is not None:
                    nc.vector.memset(gh_t[:], 0.25)

            def emit_body():
                nonlocal u_t, s_t, g_t, g2_t, gh_t
                if variant == "full":
                    if need_u:
                        u_t = work.tile([128, NT, BLOC], f16, tag="u")
                    if U_SQUARE_ON == "act":
                        s_t = work.tile([128, NT, BLOC], f16, tag="s")
                    g_t = work.tile([128, NT, BLOC], f16, tag="g")
                    g2_t = work.tile([128, NT, BLOC], f16, tag="g2")
                    gh_t = work.tile([128, NT, BLOC], f16, tag="gh")
                o1p = o2p = None
                if variant != "nope":
                    o1p = psum.tile([O, BLOC], f32, tag="o1p")
                    if variant != "pe16":
                        o2p = psum.tile([O, BLOC], f32, tag="o2p")
                if variant in ("peonly", "pe16"):
                    for tt in range(NT):
                        first = tt == 0
                        last = tt == NT - 1
                        nc.tensor.matmul(o1p[:], wg_t[:, tt], g_t[:, tt],
                                         start=first, stop=last)
                        if variant == "peonly":
                            nc.tensor.matmul(o2p[:], we_t[:, tt], g2_t[:, tt],
                                             start=first, stop=False)
                            nc.tensor.matmul(o2p[:], wo_t[:, tt], gh_t[:, tt],
                                             start=False, stop=last)
                    nc.vector.tensor_scalar_add(o1s[:], o1p[:], 0.0)
                    if variant == "peonly":
                        nc.vector.tensor_scalar_add(o2s[:], o2p[:], 0.0)
                    return
                for g in range(NG):
                    sl = slice(g * GT, (g + 1) * GT)
                    if g % ACT_GROUPS == 0:
                        # Act runs on coarser slices to amortize access latency
                        sla = slice(g * GT, (g + ACT_GROUPS) * GT)
                        sq_dve = g >= NG - SQ_DVE_LAST
                        if U_SQUARE_ON == "act" and CSPL >= BLOC and not sq_dve:
                            # s = (xs + (-z))^2 with per-partition bias
                            nc.scalar.activation(
                                s_t[:, sla], xs_t[:, sla], Square,
                                bias=zn_t[:, :1], scale=1.0,
                            )
                            src = s_t
                        elif U_SQUARE_ON == "act" and not sq_dve:
                            # column-split: first CSPL cols on Act, rest on DVE
                            nc.scalar.activation(
                                s_t[:, sla, 0:CSPL], xs_t[:, sla, 0:CSPL], Square,
                                bias=zn_t[:, :1], scale=1.0,
                            )
                            nc.vector.tensor_scalar_add(
                                u_t[:, sla, 0:BLOC - CSPL],
                                xs_t[:, sla, CSPL:BLOC], zn_t[:, :1])
                            nc.vector.tensor_mul(
                                s_t[:, sla, CSPL:BLOC],
                                u_t[:, sla, 0:BLOC - CSPL],
                                u_t[:, sla, 0:BLOC - CSPL])
                            src = s_t
                        else:
                            nc.vector.tensor_scalar_add(
                                u_t[:, sla], xs_t[:, sla], zn_t[:, :1])
                            nc.vector.tensor_mul(u_t[:, sla], u_t[:, sla],
                                                 u_t[:, sla])
                            src = u_t
                        nc.scalar.activation(g_t[:, sla], src[:, sla], Exp,
                                             scale=-1.0)
                    if PSPL >= BLOC:
                        nc.vector.tensor_mul(g2_t[:, sl], g_t[:, sl], g_t[:, sl])
                    else:
                        nc.vector.tensor_mul(g2_t[:, sl, 0:PSPL],
                                             g_t[:, sl, 0:PSPL],
                                             g_t[:, sl, 0:PSPL])
                        nc.gpsimd.tensor_mul(g2_t[:, sl, PSPL:BLOC],
                                             g_t[:, sl, PSPL:BLOC],
                                             g_t[:, sl, PSPL:BLOC])
                    if variant == "full":
                        # shifted copy on PE: gsp[k] = g[k+1] (row 127 -> 0)
                        # moving free capped at 512 per matmul (1 PSUM bank)
                        gsp = gpsum.tile([128, GT, BLOC], f32, tag="gsp")
                        hstep = max(1, 512 // BLOC)
                        for h in range(0, GT, hstep):
                            nc.tensor.matmul(
                                gsp[:, h:h + hstep], psh_t[:],
                                g_t[:, g * GT + h:g * GT + h + hstep],
                                start=True, stop=True)
                        nc.vector.tensor_mul(gh_t[:, sl], g_t[:, sl], gsp[:])
                    if variant == "nope":
                        continue
                    for tt in range(g * GT, (g + 1) * GT):
                        first = tt == 0
                        last = tt == NT - 1
                        nc.tensor.matmul(o1p[:], wg_t[:, tt], g_t[:, tt],
                                         start=first, stop=last)
                        nc.tensor.matmul(o2p[:], we_t[:, tt], g2_t[:, tt],
                                         start=first, stop=(last and variant == "nogh"))
                    if variant != "nogh":
                        for tt in range(g * GT, (g + 1) * GT):
                            nc.tensor.matmul(o2p[:], wo_t[:, tt], gh_t[:, tt],
                                             start=False, stop=(tt == NT - 1))
                if variant == "nope":
                    return
                nc.vector.tensor_scalar_add(o1s[:], o1p[:], 0.0)
                nc.vector.tensor_scalar_add(o2s[:], o2p[:], 0.0)

            if repeat == 1:
                emit_body()
            elif unroll:
                for _ in range(repeat):
                    emit_body()
            else:
                u = BENCH_UNROLL if repeat % BENCH_UNROLL == 0 else 1
                with tc.For_i(0, repeat // u, 1, staggered_reset=STAGGERED):
                    for _ in range(u):
                        emit_body()

            if o1s is not None:
                nc.sync.dma_start(out1_d.ap()[:], o1s[:])
            if o2s is not None:
                nc.sync.dma_start(out2_d.ap()[:], o2s[:])

    nc.compile()
    return nc


def _host_weights(zlin, lensq, q_mu, q_log_var, log_variance):
    """Dense fp16 weights in [k=(i4,m), t, o] layout (float64 host math)."""
    f64 = np.float64
    q_mu = np.asarray(q_mu, f64)
    q_var = np.maximum(np.exp(np.asarray(q_log_var, f64)), EPS_QVAR)
    vk = np.maximum(np.exp(np.asarray(log_variance, f64)), EPS_VAR)
    z = np.asarray(zlin, f64)
    D = f64(lensq) + EPS_XVAR
    rho = np.sqrt(f64(lensq) / D)
    c1 = vk * rho                                   # [O, I]
    w1d = c1[:, :, None] * q_mu                     # [O, I, M]
    w2d = (c1 ** 2)[:, :, None] * (q_var + q_mu ** 2)

    # W3[o,i,p], p = m+m' in 0..2M-2: pair expansion of em^2 on half grid
    delta = z[1] - z[0]
    W3 = np.zeros((O, I, 2 * M - 1), f64)
    midx = np.arange(M)
    for m in range(M):
        c = np.exp(-(delta ** 2) * ((m - midx) ** 2) / (4.0 * D))
        W3[:, :, m + midx] += w1d[:, :, m:m + 1] * w1d * c[None, None, :]
    W4even = w2d - W3[:, :, 0::2]                   # [O, I, M]
    W4odd = np.zeros((O, I, M), f64)                # pad m=31 with 0
    W4odd[:, :, :M - 1] = -W3[:, :, 1::2] * np.exp(delta ** 2 / (4.0 * D))

    def blockify(w):                                # [O,I,M] -> [128, NT, O]
        return np.ascontiguousarray(
            w.reshape(O, NT, IB, M).transpose(2, 3, 1, 0).reshape(128, NT, O)
        ).astype(np.float16)

    return blockify(w1d), blockify(W4even), blockify(W4odd)


def _host_prep(x, zlin, lensq, wg, we, wo):
    """Per-core input maps.  x is scaled by s1 on host and cast to fp16."""
    f64 = np.float64
    D = f64(lensq) + EPS_XVAR
    s1 = 1.0 / np.sqrt(2.0 * D)
    z = np.asarray(zlin, f64)
    zn = np.ascontiguousarray(
        np.tile(-s1 * z, IB).reshape(128, 1).astype(np.float32))

    ps = np.eye(128, k=-1, dtype=np.float16)            # out[k] = in[k+1]
    xs = (s1 * np.asarray(x, f64)).astype(np.float16)   # [B, I]
    in_maps = []
    for c in range(NCORES):
        xT = xs[c * BLOC:(c + 1) * BLOC].T              # [I, BLOC]
        arr = xT.reshape(NT, IB, BLOC).transpose(1, 0, 2)   # [i4, t, b]
        x4 = np.ascontiguousarray(
            np.broadcast_to(arr[:, None], (IB, M, NT, BLOC)).reshape(128, NT, BLOC)
        )
        in_maps.append({"xs": x4, "zn": zn, "wg": wg, "we": we, "wo": wo,
                        "ps": ps})
    return in_maps


def _fallback(x, z, q_mu, q_log_var, log_scale, log_variance):
    """Generic numpy implementation (mirrors the reference exactly)."""
    x = np.asarray(x, np.float32)
    q_var = np.maximum(np.exp(np.asarray(q_log_var, np.float32)), EPS_QVAR)
    var_kern = np.maximum(np.exp(np.asarray(log_variance, np.float32)), EPS_VAR)
    lengthscale = np.maximum(np.exp(np.asarray(log_scale, np.float32)), MIN_SCALE)
    ell_sq = lengthscale ** 2
    denom = ell_sq + EPS_XVAR                      # [O, I]
    rho = np.sqrt(ell_sq / denom)
    z = np.asarray(z, np.float32)
    q_mu = np.asarray(q_mu, np.float32)
    w2 = q_var + q_mu ** 2
    nb, no = x.shape[0], z.shape[0]
    o1 = np.empty((nb, no), np.float32)
    o2 = np.empty((nb, no), np.float32)
    for b0 in range(0, nb, 128):
        xs = x[b0:b0 + 128]
        diff = xs[:, None, :, None] - z[None]      # [b, O, I, M]
        psi = (var_kern * rho)[None, :, :, None] * np.exp(
            -0.5 * diff ** 2 / denom[None, :, :, None]
        )
        em = np.einsum("boim,oim->boi", psi, q_mu)
        ev = np.einsum("boim,oim->boi", psi ** 2, w2)
        o1[b0:b0 + 128] = em.sum(2)
        o2[b0:b0 + 128] = np.maximum(ev - em ** 2, EPS_EDGE).sum(2)
    return o1, o2


def _structure(x, z, q_mu, q_log_var, log_scale, log_variance):
    """Return (zlin, lensq) if the fast-path structure holds, else None."""
    if x.shape != (B, I) or z.shape != (O, I, M):
        return None
    z = np.asarray(z)
    if not (z == z[0, 0]).all():
        return None
    zlin = np.asarray(z[0, 0], np.float64)
    d = np.diff(zlin)
    if d[0] <= 0 or not np.allclose(d, d[0], rtol=1e-5, atol=1e-7):
        return None                                 # need a uniform grid
    ls = np.maximum(np.exp(np.asarray(log_scale, np.float32)), np.float32(MIN_SCALE))
    if not (ls == ls.flat[0]).all():
        return None
    return zlin, np.float32(ls.flat[0]) ** 2


def kernel(x, z, q_mu, q_log_var, log_scale, log_variance):
    st = _structure(x, z, q_mu, q_log_var, log_scale, log_variance)
    if st is None:
        return _fallback(x, z, q_mu, q_log_var, log_scale, log_variance)
    zlin, lensq = st

    wg, we, wo = _host_weights(zlin, lensq, q_mu, q_log_var, log_variance)
    in_maps = _host_prep(x, zlin, lensq, wg, we, wo)

    from concourse.bass_utils import run_bass_kernel_spmd

    if "nc" not in _NC_CACHE:
        _NC_CACHE["nc"] = _build_nc(repeat=1)
    nc = _NC_CACHE["nc"]
    res = run_bass_kernel_spmd(nc, in_maps, list(range(NCORES)))
    out1 = np.concatenate(
        [res.results[c]["out1"].T for c in range(NCORES)], 0)
    out2 = np.concatenate(
        [res.results[c]["out2"].T for c in range(NCORES)], 0)
    return np.ascontiguousarray(out1), np.ascontiguousarray(out2)
